# revision 2
# baseline (speedup 1.0000x reference)
"""Autoformer forward (nn_Autoformer_45363444580643), full-device version.

The axon tunnel is the bottleneck (~80ms fixed/leg, slow MB/s), so the
entire network runs in ONE Bass kernel on 8 cores (2 samples/core, batch
data-parallel):
- weights ship f32 (top-k delay selection needs f32-class precision: min
  rank-19/20 mean_value gap ~1e-5 of range; bf16/int16 weights flip
  delays -> 0.2+ max-rel error), SHARDED 1/8 per core + device AllGather.
- x_enc ships int16 (scale folded into token-conv weights), marks f32.
- autocorrelation mean_value via direct Gram matmul G'' = K Q^T and
  circulant-diagonal sums using strided skew-DMA reads of a doubled G
  buffer in DRAM (no FFT, f32 end-to-end).
- top-k via DVE max8/max_index/match_replace rounds; softmax + scatter
  (iota compare) builds reversed selection vector s2r; the circulant
  mixing matrix M comes from skew-DMA reads of s2r; aggregation = V^T M.
- output: int8 + per-row f32 scales (post-decision, ~0.4% err).
"""

import dataclasses
import math
from contextlib import ExitStack

import numpy as np

from scipy import fft as sfft
from scipy.special import erf as _erf

B, SEQ_LEN, LABEL_LEN, PRED_LEN = 16, 96, 48, 720
N_SERIES, D_MODEL, N_HEADS, D_FF = 321, 256, 8, 1024
E_LAYERS, D_LAYERS, MOVING_AVG, FACTOR = 2, 1, 25, 3
EPS = 1e-5
N_CORES = 8
BPC = B // N_CORES
LD = LABEL_LEN + PRED_LEN  # 768
TK_D = int(FACTOR * math.log(LD))       # 19
TK_E = int(FACTOR * math.log(SEQ_LEN))  # 13
NMRK = 4 * (SEQ_LEN + LD)

_RT = None
_RT_ERR = None


def _wlayout():
    ws = []
    for k in range(3):
        ws.append((f"enc_tok{k}", (N_SERIES, D_MODEL)))
    ws.append(("enc_time", (4, D_MODEL)))
    for l in range(E_LAYERS):
        for nm in ("q", "k", "v", "o"):
            ws.append((f"enc_a{l}{nm}", (D_MODEL, D_MODEL)))
        ws.append((f"enc_ab{l}", (4, D_MODEL)))
        ws.append((f"enc_f1{l}", (D_MODEL, D_FF)))
        ws.append((f"enc_f2{l}", (D_FF, D_MODEL)))
    ws.append(("enc_lnw", (1, D_MODEL)))
    ws.append(("enc_lnb", (1, D_MODEL)))
    for k in range(3):
        ws.append((f"dec_tok{k}", (N_SERIES, D_MODEL)))
    ws.append(("dec_time", (4, D_MODEL)))
    for nm in ("q", "k", "v", "o"):
        ws.append((f"dec_s{nm}", (D_MODEL, D_MODEL)))
    ws.append(("dec_sb", (4, D_MODEL)))
    for nm in ("q", "k", "v", "o"):
        ws.append((f"dec_c{nm}", (D_MODEL, D_MODEL)))
    ws.append(("dec_cb", (4, D_MODEL)))
    ws.append(("dec_f1", (D_MODEL, D_FF)))
    ws.append(("dec_f2", (D_FF, D_MODEL)))
    for k in range(3):
        ws.append((f"dec_tr{k}", (D_MODEL, N_SERIES)))
    ws.append(("proj", (D_MODEL, N_SERIES)))
    ws.append(("proj_b", (1, N_SERIES)))
    ws.append(("dec_lnw", (1, D_MODEL)))
    ws.append(("dec_lnb", (1, D_MODEL)))
    offs = {}
    o = 0
    for nm, sh in ws:
        offs[nm] = (o, sh)
        o += int(np.prod(sh))
    quant = N_CORES * 4096
    return offs, ((o + quant - 1) // quant) * quant


W_OFFS, NW = _wlayout()


def _build_M(L, k):
    p = (k - 1) // 2
    M = np.zeros((L, L), np.float32)
    for t in range(L):
        for s in range(t - p, t + p + 1):
            u = min(max(s, 0), L - 1)
            M[t, u] += 1.0 / k
    return M


def _build_nc(num_devices=N_CORES):
    import concourse.mybir as mybir
    import concourse.tile as tile
    from concourse import bacc

    f32 = mybir.dt.float32
    i16 = mybir.dt.int16
    i8 = mybir.dt.int8
    i32 = mybir.dt.int32
    u32 = mybir.dt.uint32
    G = mybir.ActivationFunctionType
    A = mybir.AluOpType
    X = mybir.AxisListType.X

    nc = bacc.Bacc("TRN2", target_bir_lowering=False, debug=False,
                   enable_asserts=False, num_devices=num_devices)

    wsh_d = nc.dram_tensor("wsh", [NW // N_CORES], f32, kind="ExternalInput")
    xe_d = nc.dram_tensor("xe", [BPC, SEQ_LEN, N_SERIES], i16,
                          kind="ExternalInput")
    ab_d = nc.dram_tensor("ab", [16 + BPC * NMRK], f32, kind="ExternalInput")
    out_d = nc.dram_tensor("out", [BPC, PRED_LEN, N_SERIES], i8,
                           kind="ExternalOutput")
    osc_d = nc.dram_tensor("osc", [BPC, PRED_LEN], f32, kind="ExternalOutput")

    wst_d = nc.dram_tensor("wstage", [NW // N_CORES], f32, kind="Internal")
    wf_d = nc.dram_tensor("wfull", [NW], f32, kind="Internal",
                          addr_space="Shared")
    gd_d = nc.dram_tensor("gselfbuf", [BPC, LD, 2 * LD], f32, kind="Internal")
    gc_d = nc.dram_tensor("gcrossbuf", [BPC, SEQ_LEN, 2 * LD], f32,
                          kind="Internal")
    ge_d = nc.dram_tensor("gencbuf", [BPC, SEQ_LEN, 2 * SEQ_LEN], f32,
                          kind="Internal")
    sd_d = nc.dram_tensor("s2rself", [BPC, 2 * LD], f32, kind="Internal")
    sc_d = nc.dram_tensor("s2rcross", [BPC, 2 * LD], f32, kind="Internal")
    se_d = nc.dram_tensor("s2renc", [BPC, 2 * SEQ_LEN], f32, kind="Internal")

    MT768_d = nc.inline_tensor(_build_M(LD, MOVING_AVG).T.copy(), name="MT768")
    MA96_d = nc.inline_tensor(_build_M(SEQ_LEN, MOVING_AVG).T.copy(),
                              name="MA96")
    id_d = nc.inline_tensor(np.eye(128, dtype=np.float32), name="idc")

    def dap(handle, offset, dims):
        return dataclasses.replace(handle.ap(), offset=offset,
                                   ap=[list(d) for d in dims])

    CB321 = [(0, 128), (128, 128), (256, 65)]

    with tile.TileContext(nc) as tc, ExitStack() as ctx:
        wp = ctx.enter_context(tc.tile_pool(name="w", bufs=1))
        sp = ctx.enter_context(tc.tile_pool(name="s", bufs=1))
        tp = ctx.enter_context(tc.tile_pool(name="t", bufs=1))
        pbig = ctx.enter_context(tc.tile_pool(name="pb", bufs=3, space="PSUM"))
        ptr = ctx.enter_context(tc.tile_pool(name="pt", bufs=2, space="PSUM"))
        pout = ctx.enter_context(tc.tile_pool(name="po", bufs=1, space="PSUM"))
        pmv = ctx.enter_context(tc.tile_pool(name="pm", bufs=2, space="PSUM"))

        nc.sync.dma_start(
            wst_d.ap().rearrange("(a b) -> a b", b=4096),
            wsh_d.ap().rearrange("(a b) -> a b", b=4096))
        nc.gpsimd.collective_compute(
            "AllGather", mybir.AluOpType.bypass,
            replica_groups=[list(range(N_CORES))],
            ins=[wst_d.ap()], outs=[wf_d.ap()])

        def wload(nm, pool, tag):
            off, (P, F) = W_OFFS[nm]
            out = []
            p0 = 0
            bi = 0
            while p0 < P:
                pw = min(128, P - p0)
                t = pool.tile([pw, F], f32, tag=f"{tag}_{bi}",
                              name=f"{tag}_{bi}")
                nc.sync.dma_start(
                    t[:], wf_d.ap()[off + p0 * F: off + (p0 + pw) * F]
                    .rearrange("(p f) -> p f", p=pw))
                out.append(t)
                p0 += pw
                bi += 1
            return out

        def bload(nm, tag):
            off, (P, F) = W_OFFS[nm]
            rows = []
            for r in range(P):
                t = tp.tile([1, F], f32, tag=f"{tag}_{r}", name=f"{tag}_{r}")
                nc.sync.dma_start(
                    t[:], wf_d.ap()[None, off + r * F: off + (r + 1) * F])
                rows.append(t)
            return rows

        # constants
        ident = wp.tile([128, 128], f32, tag="ident", name="ident")
        nc.sync.dma_start(ident[:], id_d.ap())
        MA96 = wp.tile([SEQ_LEN, SEQ_LEN], f32, tag="MA96", name="MA96")
        nc.sync.dma_start(MA96[:], MA96_d.ap())
        onesc = wp.tile([128, 1], f32, tag="onesc", name="onesc")
        nc.vector.memset(onesc[:], 1.0)
        onesr = wp.tile([1, LD], f32, tag="onesr", name="onesr")
        nc.vector.memset(onesr[:], 1.0)
        ones1 = wp.tile([1, 128], f32, tag="ones1", name="ones1")
        nc.vector.memset(ones1[:], 1.0)
        eps_t = wp.tile([1, 1], f32, tag="eps", name="eps")
        nc.vector.memset(eps_t[:], EPS)
        ioL_i = tp.tile([32, 2 * LD], i32, tag="teq", name="ioLi")
        nc.gpsimd.iota(ioL_i[:], pattern=[[0, 2], [1, LD]], base=0,
                       channel_multiplier=0)
        ioL = wp.tile([32, 2 * LD], f32, tag="ioL", name="ioL")
        nc.vector.tensor_copy(ioL[:], ioL_i[:])
        ioS_i = tp.tile([32, 2 * SEQ_LEN], i32, tag="ts2", name="ioSi")
        nc.gpsimd.iota(ioS_i[:], pattern=[[0, 2], [1, SEQ_LEN]], base=0,
                       channel_multiplier=0)
        ioS = wp.tile([32, 2 * SEQ_LEN], f32, tag="ioS", name="ioS")
        nc.vector.tensor_copy(ioS[:], ioS_i[:])
        hdr = wp.tile([1, 16], f32, tag="hdr", name="hdr")
        nc.sync.dma_start(hdr[:], ab_d.ap()[None, 0:16])
        marks = []
        for s in range(BPC):
            m = wp.tile([4, SEQ_LEN + LD], f32, tag=f"marks{s}",
                        name=f"marks{s}")
            nc.sync.dma_start(
                m[:], ab_d.ap()[16 + s * NMRK: 16 + (s + 1) * NMRK]
                .rearrange("(p f) -> p f", p=4))
            marks.append(m)

        # MT768 loader (on-demand into the shared sl768 slots)
        def mt_tile(t):
            mt = tp.tile([128, LD], f32, tag=f"sl768_{t}", name=f"mt_{t}")
            nc.sync.dma_start(mt[:], MT768_d.ap()[t * 128:(t + 1) * 128])
            return mt

        # ---------------- encoder input prep
        xeT, seasT, arow = [], [], []
        for s in range(BPC):
            xi = tp.tile([SEQ_LEN, N_SERIES], i16, tag="xei", name="xei")
            nc.sync.dma_start(xi[:], xe_d.ap()[s])
            xn = tp.tile([SEQ_LEN, N_SERIES], f32, tag="xen", name="xen")
            nc.vector.tensor_copy(xn[:], xi[:])
            xt = [wp.tile([pw, SEQ_LEN], f32, tag=f"xeT{s}_{p0}",
                          name=f"xeT{s}_{p0}") for p0, pw in CB321]
            st_ = [wp.tile([pw, SEQ_LEN], f32, tag=f"seasT{s}_{p0}",
                           name=f"seasT{s}_{p0}") for p0, pw in CB321]
            ar = wp.tile([1, N_SERIES], f32, tag=f"arow{s}", name=f"arow{s}")
            rm = tp.tile([128, 1], f32, tag="rmean", name="rmean")
            for bi, (p0, pw) in enumerate(CB321):
                pst = ptr.tile([128, 321], f32, tag="pst", name="pst")
                nc.tensor.transpose(pst[0:pw, 0:SEQ_LEN], xn[:, p0:p0 + pw],
                                    ident[0:SEQ_LEN, 0:SEQ_LEN])
                nc.vector.tensor_copy(xt[bi][:], pst[0:pw, 0:SEQ_LEN])
                ps = pbig.tile([128, 512], f32, tag="ps", name="ps")
                nc.tensor.matmul(ps[0:pw, 0:SEQ_LEN], xn[:, p0:p0 + pw],
                                 MA96[:], start=True, stop=True)
                nc.vector.tensor_sub(st_[bi][:], xt[bi][:],
                                     ps[0:pw, 0:SEQ_LEN])
                nc.vector.tensor_reduce(rm[0:pw, :], xt[bi][:], X, A.add)
                pst2 = ptr.tile([128, 321], f32, tag="pst", name="pst2")
                nc.tensor.transpose(pst2[0:1, 0:pw], rm[0:pw, :],
                                    ident[0:pw, 0:pw])
                nc.scalar.activation(ar[:, p0:p0 + pw], pst2[0:1, 0:pw],
                                     G.Copy, scale=1.0 / SEQ_LEN)
            nc.vector.tensor_scalar_mul(ar[:], ar[:], hdr[:, 0:1])
            xeT.append(xt)
            seasT.append(st_)
            arow.append(ar)
        pjb = wload("proj_b", wp, "pjb")
        for s in range(BPC):
            nc.vector.tensor_add(arow[s][:], arow[s][:], pjb[0][:])

        xeTp = []
        for s in range(BPC):
            xp = [wp.tile([pw, SEQ_LEN + 2], f32, tag=f"xeTp{s}_{p0}",
                          name=f"xeTp{s}_{p0}") for p0, pw in CB321]
            for bi in range(3):
                nc.vector.tensor_copy(xp[bi][:, 1:SEQ_LEN + 1], xeT[s][bi][:])
                nc.vector.tensor_copy(xp[bi][:, 0:1],
                                      xeT[s][bi][:, SEQ_LEN - 1:SEQ_LEN])
                nc.vector.tensor_copy(xp[bi][:, SEQ_LEN + 1:SEQ_LEN + 2],
                                      xeT[s][bi][:, 0:1])
            xeTp.append(xp)

        FE = BPC * SEQ_LEN  # 192

        # ---------------- shared helpers

        def rev_free(ap_, n):
            return dataclasses.replace(
                ap_, offset=ap_.offset + n - 1, ap=[ap_.ap[0], [-1, n]])

        def topk_scatter(mvt, Lx, tk, s2r_ap, io_f):
            """mvt: [1, Lx] f32 (destroyed). Writes s2r row (len 2*Lx,
            value s_vec[(Lx-1-j) mod Lx]) to s2r_ap."""
            R = (tk + 7) // 8
            NK = R * 8
            vals = tp.tile([1, 24], f32, tag="tv", name="tv")
            idxs = tp.tile([1, 24], u32, tag="ti", name="ti")
            for r in range(R):
                nc.vector.max(vals[:, r * 8:(r + 1) * 8], mvt[:])
                nc.vector.max_index(idxs[:, r * 8:(r + 1) * 8],
                                    vals[:, r * 8:(r + 1) * 8], mvt[:])
                if r < R - 1:
                    nc.vector.match_replace(
                        mvt[:], vals[:, r * 8:(r + 1) * 8], mvt[:], -1e30)
            idxf = tp.tile([1, 24], f32, tag="tif", name="tif")
            nc.vector.tensor_copy(idxf[:, 0:NK], idxs[:, 0:NK])
            w = tp.tile([1, 24], f32, tag="tw", name="tw")
            ngm = tp.tile([1, 1], f32, tag="tn", name="tn")
            nc.vector.tensor_scalar_mul(ngm[:], vals[:, 0:1], -1.0)
            nc.scalar.activation(w[:, 0:tk], vals[:, 0:tk], G.Exp,
                                 bias=ngm[:])
            wsm = tp.tile([1, 1], f32, tag="tsm", name="tsm")
            nc.vector.tensor_reduce(wsm[:], w[:, 0:tk], X, A.add)
            nc.vector.reciprocal(wsm[:], wsm[:])
            nc.vector.tensor_scalar_mul(w[:, 0:tk], w[:, 0:tk], wsm[:])
            epad = tp.tile([32, 32], f32, tag="tep", name="tep")
            wpad = tp.tile([32, 32], f32, tag="twp", name="twp")
            nc.vector.memset(epad[:], -7.0)
            nc.vector.memset(wpad[:], 0.0)
            nc.vector.tensor_copy(epad[0:1, 0:NK], idxf[:, 0:NK])
            nc.vector.tensor_copy(wpad[0:1, 0:tk], w[:, 0:tk])
            et = tp.tile([32, 32], f32, tag="tet", name="tet")
            wt = tp.tile([32, 32], f32, tag="twt", name="twt")
            nc.vector.transpose(et[:], epad[:])
            nc.vector.transpose(wt[:], wpad[:])
            eqm = tp.tile([32, 2 * LD], f32, tag="teq", name="teq")
            nc.vector.tensor_scalar(eqm[:, 0:2 * Lx], io_f[0:32, :],
                                    et[0:32, 0:1], None, A.is_equal)
            nc.vector.tensor_scalar_mul(eqm[:, 0:2 * Lx], eqm[:, 0:2 * Lx],
                                        wt[0:32, 0:1])
            s2r = tp.tile([1, 2 * LD], f32, tag="ts2", name="ts2")
            nh = (2 * Lx + 511) // 512
            for h in range(nh):
                c0 = h * 512
                cw = min(512, 2 * Lx - c0)
                ps = pmv.tile([BPC, 512], f32, tag="pm", name="pms2")
                nc.tensor.matmul(ps[0:1, 0:cw], onesc[0:32, :],
                                 eqm[0:32, c0:c0 + cw], start=True,
                                 stop=True)
                nc.vector.tensor_copy(s2r[:, c0:c0 + cw], ps[0:1, 0:cw])
            nc.sync.dma_start(s2r_ap, s2r[:, 0:2 * Lx])

        def dec_decomp(xin, xout, tacc, mode):
            mt_cache = {t: mt_tile(t) for t in range(6)}
            xn = [tp.tile([128, D_MODEL], f32, tag=f"avn_{t}",
                          name=f"dxn_{t}") for t in range(6)]
            for t in range(6):
                for cb in range(2):
                    pst = ptr.tile([128, 321], f32, tag="pst", name="pstd")
                    nc.tensor.transpose(pst[:, 0:128],
                                        xin[cb][:, t * 128:(t + 1) * 128],
                                        ident[:])
                    nc.vector.tensor_copy(xn[t][:, cb * 128:(cb + 1) * 128],
                                          pst[:, 0:128])
            for cb in range(2):
                for h in range(2):
                    ps = pbig.tile([128, 512], f32, tag="ps", name="psdd")
                    for t in range(6):
                        mt = mt_cache[t]
                        nc.tensor.matmul(ps[:, 0:384],
                                         xn[t][:, cb * 128:(cb + 1) * 128],
                                         mt[:, h * 384:(h + 1) * 384],
                                         start=(t == 0), stop=(t == 5))
                    sl = slice(h * 384, (h + 1) * 384)
                    nc.vector.tensor_sub(xout[cb][:, sl], xin[cb][:, sl],
                                         ps[:, 0:384])
                    if tacc is not None:
                        if mode == "add":
                            nc.vector.tensor_add(tacc[cb][:, sl],
                                                 tacc[cb][:, sl], ps[:, 0:384])
                        else:
                            nc.vector.tensor_copy(tacc[cb][:, sl], ps[:, 0:384])

        def mylayernorm(xT, F, lnw_row, lnb_row, seq_ranges):
            nh = (F + 383) // 384
            mu = tp.tile([1, LD], f32, tag="lnmu", name="lnmu")
            for h in range(nh):
                c0 = h * 384
                cw = min(384, F - c0)
                ps = pmv.tile([BPC, 512], f32, tag="pm", name="psmu")
                for cb in range(2):
                    nc.tensor.matmul(ps[0:1, 0:cw], onesc[:],
                                     xT[cb][:, c0:c0 + cw],
                                     start=(cb == 0), stop=(cb == 1))
                nc.scalar.activation(mu[:, c0:c0 + cw], ps[0:1, 0:cw],
                                     G.Copy, scale=1.0 / D_MODEL)
            mub = tp.tile([128, LD], f32, tag="sl768_6", name="lnmub")
            for h in range(nh):
                c0 = h * 384
                cw = min(384, F - c0)
                ps = pbig.tile([128, 512], f32, tag="ps", name="psmub")
                nc.tensor.matmul(ps[:, 0:cw], ones1[:], mu[:, c0:c0 + cw],
                                 start=True, stop=True)
                nc.vector.tensor_copy(mub[:, c0:c0 + cw], ps[:, 0:cw])
            for cb in range(2):
                nc.vector.tensor_sub(xT[cb][:, 0:F], xT[cb][:, 0:F],
                                     mub[:, 0:F])
            sq = tp.tile([128, LD], f32, tag="sl768_7", name="lnsq")
            sd = tp.tile([1, LD], f32, tag="lnsd", name="lnsd")
            for h in range(nh):
                c0 = h * 384
                cw = min(384, F - c0)
                ps = pmv.tile([BPC, 512], f32, tag="pm", name="psvar")
                for cb in range(2):
                    nc.scalar.activation(sq[:, c0:c0 + cw],
                                         xT[cb][:, c0:c0 + cw], G.Square)
                    nc.tensor.matmul(ps[0:1, 0:cw], onesc[:],
                                     sq[:, c0:c0 + cw],
                                     start=(cb == 0), stop=(cb == 1))
                nc.scalar.activation(sd[:, c0:c0 + cw], ps[0:1, 0:cw],
                                     G.Sqrt, scale=1.0 / D_MODEL,
                                     bias=eps_t[:])
            nc.vector.reciprocal(sd[:, 0:F], sd[:, 0:F])
            rsb = tp.tile([128, LD], f32, tag="sl768_5", name="lnrs")
            for h in range(nh):
                c0 = h * 384
                cw = min(384, F - c0)
                ps = pbig.tile([128, 512], f32, tag="ps", name="psrsb")
                nc.tensor.matmul(ps[:, 0:cw], ones1[:], sd[:, c0:c0 + cw],
                                 start=True, stop=True)
                nc.vector.tensor_copy(rsb[:, c0:c0 + cw], ps[:, 0:cw])
            # ln w/b columns via transpose of the [1, 256] rows
            wc = tp.tile([128, 2], f32, tag="lnwc", name="lnwc")
            bc = tp.tile([128, 2], f32, tag="lnbc", name="lnbc")
            for cb in range(2):
                pst = ptr.tile([128, 321], f32, tag="pst", name="pstln")
                nc.tensor.transpose(pst[0:128, 0:1],
                                    lnw_row[0:1, cb * 128:(cb + 1) * 128],
                                    ident[0:1, 0:1])
                nc.vector.tensor_copy(wc[:, cb:cb + 1], pst[0:128, 0:1])
                pst2 = ptr.tile([128, 321], f32, tag="pst", name="pstln2")
                nc.tensor.transpose(pst2[0:128, 0:1],
                                    lnb_row[0:1, cb * 128:(cb + 1) * 128],
                                    ident[0:1, 0:1])
                nc.vector.tensor_copy(bc[:, cb:cb + 1], pst2[0:128, 0:1])
            for cb in range(2):
                nc.vector.tensor_mul(xT[cb][:, 0:F], xT[cb][:, 0:F],
                                     rsb[:, 0:F])
                nc.vector.tensor_scalar(xT[cb][:, 0:F], xT[cb][:, 0:F],
                                        wc[:, cb:cb + 1], bc[:, cb:cb + 1],
                                        A.mult, A.add)
            for s0, sl_ in seq_ranges:
                for cb in range(2):
                    rm = tp.tile([128, 1], f32, tag="lnrm", name="lnrm")
                    nc.vector.tensor_reduce(rm[:], xT[cb][:, s0:s0 + sl_], X,
                                            A.add)
                    nc.vector.tensor_scalar_mul(rm[:], rm[:], 1.0 / sl_)
                    nc.vector.tensor_scalar_sub(xT[cb][:, s0:s0 + sl_],
                                                xT[cb][:, s0:s0 + sl_], rm[:])

        # ---------------- encoder embedding
        etok = [wload(f"enc_tok{k}", tp, f"tok{k}") for k in range(3)]
        etim = wload("enc_time", tp, "tim")
        encT = [wp.tile([128, FE], f32, tag=f"encT{j}", name=f"encT{j}")
                for j in range(2)]
        for j in range(2):
            ps = pbig.tile([128, 512], f32, tag="ps", name="psemb")
            for s in range(BPC):
                n0 = s * SEQ_LEN
                first = True
                for k in range(3):
                    for bi in range(3):
                        nc.tensor.matmul(
                            ps[:, n0:n0 + SEQ_LEN],
                            etok[k][bi][:, j * 128:(j + 1) * 128],
                            xeTp[s][bi][:, k:k + SEQ_LEN],
                            start=first, stop=False)
                        first = False
                nc.tensor.matmul(ps[:, n0:n0 + SEQ_LEN],
                                 etim[0][:, j * 128:(j + 1) * 128],
                                 marks[s][:, 0:SEQ_LEN], start=False,
                                 stop=True)
            nc.vector.tensor_copy(encT[j][:], ps[:, 0:FE])

        # ---------------- encoder layers
        for l in range(E_LAYERS):
            wq = wload(f"enc_a{l}q", tp, "wq")
            wk = wload(f"enc_a{l}k", tp, "wk")
            wv = wload(f"enc_a{l}v", tp, "wv")
            wo = wload(f"enc_a{l}o", tp, "wo")
            brow = bload(f"enc_ab{l}", "ab")
            qT = [tp.tile([128, LD], f32, tag=f"sl768_{j}", name=f"aqT_{j}")
                  for j in range(2)]
            kT = [tp.tile([128, LD], f32, tag=f"sl768_{2 + j}",
                          name=f"akT_{j}") for j in range(2)]
            for dst, wm, bi_ in ((qT, wq, 0), (kT, wk, 1)):
                for j in range(2):
                    ps = pbig.tile([128, 512], f32, tag="ps", name="psqk")
                    for cb in range(2):
                        nc.tensor.matmul(ps[:, 0:FE],
                                         wm[cb][:, j * 128:(j + 1) * 128],
                                         encT[cb][:], start=(cb == 0),
                                         stop=False)
                    nc.tensor.matmul(ps[:, 0:FE],
                                     brow[bi_][:, j * 128:(j + 1) * 128],
                                     onesr[:, 0:FE], start=False, stop=True)
                    nc.vector.tensor_copy(dst[j][:, 0:FE], ps[:, 0:FE])
            vn = [tp.tile([128, D_MODEL], f32, tag=f"avn_{t}",
                          name=f"avn_{t}") for t in range(6)]
            for s in range(BPC):
                ps = pbig.tile([128, 512], f32, tag="ps", name="psv")
                n0 = s * SEQ_LEN
                for cb in range(2):
                    nc.tensor.matmul(ps[0:SEQ_LEN, 0:D_MODEL],
                                     encT[cb][:, n0:n0 + SEQ_LEN], wv[cb][:],
                                     start=(cb == 0), stop=False)
                nc.tensor.matmul(ps[0:SEQ_LEN, 0:D_MODEL],
                                 onesr[:, 0:SEQ_LEN], brow[2][:],
                                 start=False, stop=True)
                nc.vector.tensor_copy(vn[s][0:SEQ_LEN, :],
                                      ps[0:SEQ_LEN, 0:D_MODEL])
            for s in range(BPC):
                n0 = s * SEQ_LEN
                ps = pbig.tile([128, 512], f32, tag="ps", name="psge")
                for cb in range(2):
                    nc.tensor.matmul(ps[0:SEQ_LEN, 0:SEQ_LEN],
                                     kT[cb][:, n0:n0 + SEQ_LEN],
                                     qT[cb][:, n0:n0 + SEQ_LEN],
                                     start=(cb == 0), stop=(cb == 1))
                gs = tp.tile([128, LD], f32, tag="ags", name="ags")
                nc.vector.tensor_copy(gs[0:SEQ_LEN, 0:SEQ_LEN],
                                      ps[0:SEQ_LEN, 0:SEQ_LEN])
                nc.sync.dma_start(ge_d.ap()[s, :, 0:SEQ_LEN],
                                  gs[0:SEQ_LEN, 0:SEQ_LEN])
                nc.sync.dma_start(ge_d.ap()[s, :, SEQ_LEN:2 * SEQ_LEN],
                                  gs[0:SEQ_LEN, 0:SEQ_LEN])
                dm = tp.tile([128, LD], f32, tag="adm", name="adm")
                nc.sync.dma_start(
                    dm[0:SEQ_LEN, 0:SEQ_LEN],
                    dap(ge_d, s * SEQ_LEN * 2 * SEQ_LEN,
                        [[2 * SEQ_LEN + 1, SEQ_LEN], [1, SEQ_LEN]]))
                psm = pmv.tile([BPC, 512], f32, tag="pm", name="psmE")
                nc.tensor.matmul(psm[0:1, 0:SEQ_LEN], onesc[0:SEQ_LEN, :],
                                 dm[0:SEQ_LEN, 0:SEQ_LEN], start=True,
                                 stop=True)
                mvE = tp.tile([1, LD], f32, tag="amv", name="amv")
                nc.scalar.activation(mvE[:, 0:SEQ_LEN],
                                     psm[0:1, 0:SEQ_LEN], G.Copy,
                                     scale=1.0 / D_MODEL)
                topk_scatter(mvE[:, 0:SEQ_LEN], SEQ_LEN, TK_E,
                             se_d.ap()[s:s + 1], ioS)
            aggT = [tp.tile([128, LD], f32, tag=f"aagT_{j}",
                            name=f"aagT_{j}") for j in range(2)]
            for s in range(BPC):
                Ms = tp.tile([128, LD], f32, tag="sl768_0", name="MsE")
                nc.sync.dma_start(
                    Ms[0:SEQ_LEN, 0:SEQ_LEN],
                    dap(se_d, s * 2 * SEQ_LEN + 1,
                        [[1, SEQ_LEN], [1, SEQ_LEN]]))
                for j in range(2):
                    ps = pbig.tile([128, 512], f32, tag="ps", name="psag")
                    nc.tensor.matmul(ps[:, 0:SEQ_LEN],
                                     vn[s][0:SEQ_LEN,
                                           j * 128:(j + 1) * 128],
                                     Ms[0:SEQ_LEN, 0:SEQ_LEN], start=True,
                                     stop=True)
                    nc.vector.tensor_copy(
                        aggT[j][:, s * SEQ_LEN:(s + 1) * SEQ_LEN],
                        rev_free(ps[:, 0:SEQ_LEN], SEQ_LEN))
            x1e = [sp.tile([128, LD + 2], f32, tag=f"xa{j}", name=f"xa{j}")
                   for j in range(2)]
            for j in range(2):
                ps = pbig.tile([128, 512], f32, tag="ps", name="psop")
                for cb in range(2):
                    nc.tensor.matmul(ps[:, 0:FE],
                                     wo[cb][:, j * 128:(j + 1) * 128],
                                     aggT[cb][:, 0:FE], start=(cb == 0),
                                     stop=False)
                nc.tensor.matmul(ps[:, 0:FE],
                                 brow[3][:, j * 128:(j + 1) * 128],
                                 onesr[:, 0:FE], start=False, stop=True)
                nc.vector.tensor_add(x1e[j][:, 0:FE], ps[:, 0:FE],
                                     encT[j][:])

            def enc_decomp(xin, xout):
                xn = [tp.tile([128, D_MODEL], f32, tag=f"avn_{s_}",
                              name=f"exn_{s_}") for s_ in range(BPC)]
                for s_ in range(BPC):
                    for cb in range(2):
                        pst = ptr.tile([128, 321], f32, tag="pst",
                                       name="pste")
                        nc.tensor.transpose(
                            pst[0:SEQ_LEN, 0:128],
                            xin[cb][:, s_ * SEQ_LEN:(s_ + 1) * SEQ_LEN],
                            ident[:])
                        nc.vector.tensor_copy(
                            xn[s_][0:SEQ_LEN, cb * 128:(cb + 1) * 128],
                            pst[0:SEQ_LEN, 0:128])
                for cb in range(2):
                    ps = pbig.tile([128, 512], f32, tag="ps", name="psed")
                    for s_ in range(BPC):
                        nc.tensor.matmul(
                            ps[:, s_ * SEQ_LEN:(s_ + 1) * SEQ_LEN],
                            xn[s_][0:SEQ_LEN, cb * 128:(cb + 1) * 128],
                            MA96[:], start=True, stop=True)
                    nc.vector.tensor_sub(xout[cb][:, 0:FE], xin[cb][:, 0:FE],
                                         ps[:, 0:FE])

            xs = [sp.tile([128, LD + 2], f32, tag=f"xb{j}", name=f"xb{j}")
                  for j in range(2)]
            enc_decomp(x1e, xs)
            w1 = wload(f"enc_f1{l}", tp, "w1")
            w2 = wload(f"enc_f2{l}", tp, "w2")
            y1 = [tp.tile([128, LD], f32, tag=f"sl768_{m}", name=f"y1E_{m}")
                  for m in range(8)]
            for m in range(8):
                ps = pbig.tile([128, 512], f32, tag="ps", name="psf1")
                for cb in range(2):
                    nc.tensor.matmul(ps[:, 0:FE],
                                     w1[cb][:, m * 128:(m + 1) * 128],
                                     xs[cb][:, 0:FE], start=(cb == 0),
                                     stop=(cb == 1))
                nc.scalar.activation(y1[m][:, 0:FE], ps[:, 0:FE], G.Gelu)
            x2 = [sp.tile([128, LD + 2], f32, tag=f"xc{j}", name=f"xc{j}")
                  for j in range(2)]
            for j in range(2):
                ps = pbig.tile([128, 512], f32, tag="ps", name="psf2")
                for m in range(8):
                    nc.tensor.matmul(ps[:, 0:FE],
                                     w2[m][:, j * 128:(j + 1) * 128],
                                     y1[m][:, 0:FE], start=(m == 0),
                                     stop=(m == 7))
                nc.vector.tensor_add(x2[j][:, 0:FE], ps[:, 0:FE],
                                     xs[j][:, 0:FE])
            enc_decomp(x2, encT)

        lnwE = wload("enc_lnw", tp, "lnw")[0]
        lnbE = wload("enc_lnb", tp, "lnb")[0]
        mylayernorm(encT, FE, lnwE, lnbE,
                    [(s * SEQ_LEN, SEQ_LEN) for s in range(BPC)])

        # ---------------- decoder

        def acl(xq, kv_enc, s, wpre, bname, gbuf, s2rbuf):
            wq = wload(f"{wpre}q", tp, "wq")
            wk = wload(f"{wpre}k", tp, "wk")
            wv = wload(f"{wpre}v", tp, "wv")
            wo = wload(f"{wpre}o", tp, "wo")
            brow = bload(bname, "ab")
            S = SEQ_LEN if kv_enc else LD
            qT = [tp.tile([128, LD], f32, tag=f"sl768_{j}", name=f"dqT_{j}")
                  for j in range(2)]
            for j in range(2):
                for h in range(2):
                    ps = pbig.tile([128, 512], f32, tag="ps", name="psdq")
                    for cb in range(2):
                        nc.tensor.matmul(ps[:, 0:384],
                                         wq[cb][:, j * 128:(j + 1) * 128],
                                         xq[cb][:, h * 384:(h + 1) * 384],
                                         start=(cb == 0), stop=False)
                    nc.tensor.matmul(ps[:, 0:384],
                                     brow[0][:, j * 128:(j + 1) * 128],
                                     onesr[:, h * 384:(h + 1) * 384],
                                     start=False, stop=True)
                    nc.vector.tensor_copy(qT[j][:, h * 384:(h + 1) * 384],
                                          ps[:, 0:384])
            kT = [tp.tile([128, LD], f32, tag=f"sl768_{2 + j}",
                          name=f"dkT_{j}") for j in range(2)]
            for j in range(2):
                for h in range((S + 383) // 384):
                    c0 = h * 384
                    cw = min(384, S - c0)
                    ps = pbig.tile([128, 512], f32, tag="ps", name="psdk")
                    for cb in range(2):
                        rhs = (encT[cb][:, s * SEQ_LEN + c0:
                                        s * SEQ_LEN + c0 + cw]
                               if kv_enc else xq[cb][:, c0:c0 + cw])
                        nc.tensor.matmul(ps[:, 0:cw],
                                         wk[cb][:, j * 128:(j + 1) * 128],
                                         rhs, start=(cb == 0), stop=False)
                    nc.tensor.matmul(ps[:, 0:cw],
                                     brow[1][:, j * 128:(j + 1) * 128],
                                     onesr[:, 0:cw], start=False, stop=True)
                    nc.vector.tensor_copy(kT[j][:, c0:c0 + cw], ps[:, 0:cw])
            nvb = (S + 127) // 128
            vn = [tp.tile([128, D_MODEL], f32, tag=f"avn_{t}",
                          name=f"dvn_{t}") for t in range(nvb)]
            for t in range(nvb):
                pw = min(128, S - t * 128)
                ps = pbig.tile([128, 512], f32, tag="ps", name="psdv")
                for cb in range(2):
                    lhs = (encT[cb][:, s * SEQ_LEN:s * SEQ_LEN + pw]
                           if kv_enc else xq[cb][:, t * 128:t * 128 + pw])
                    nc.tensor.matmul(ps[0:pw, 0:D_MODEL], lhs, wv[cb][:],
                                     start=(cb == 0), stop=False)
                nc.tensor.matmul(ps[0:pw, 0:D_MODEL], onesr[:, 0:pw],
                                 brow[2][:], start=False, stop=True)
                nc.vector.tensor_copy(vn[t][0:pw, :], ps[0:pw, 0:D_MODEL])
            # G'' and mean_value
            psm = [pmv.tile([BPC, 512], f32, tag="pm", name=f"psdm{h}")
                   for h in range(2)]
            for rb in range(nvb):
                pw = min(128, S - rb * 128)
                gs = tp.tile([128, LD], f32, tag="ags", name="dgs")
                for h in range(2):
                    ps = pbig.tile([128, 512], f32, tag="ps", name="psdg")
                    for cb in range(2):
                        nc.tensor.matmul(ps[0:pw, 0:384],
                                         kT[cb][:, rb * 128:rb * 128 + pw],
                                         qT[cb][:, h * 384:(h + 1) * 384],
                                         start=(cb == 0), stop=(cb == 1))
                    nc.vector.tensor_copy(gs[0:pw, h * 384:(h + 1) * 384],
                                          ps[0:pw, 0:384])
                nc.sync.dma_start(gbuf.ap()[s, rb * 128:rb * 128 + pw, 0:LD],
                                  gs[0:pw, :])
                nc.sync.dma_start(gbuf.ap()[s, rb * 128:rb * 128 + pw,
                                            LD:2 * LD], gs[0:pw, :])
                dm = tp.tile([128, LD], f32, tag="adm", name="ddm")
                nc.sync.dma_start(
                    dm[0:pw, :],
                    dap(gbuf, (s * S + rb * 128) * 2 * LD + rb * 128,
                        [[2 * LD + 1, pw], [1, LD]]))
                for h in range(2):
                    nc.tensor.matmul(psm[h][0:1, 0:384], onesc[0:pw, :],
                                     dm[0:pw, h * 384:(h + 1) * 384],
                                     start=(rb == 0), stop=(rb == nvb - 1))
            mvt = tp.tile([1, LD], f32, tag="amv", name="dmv")
            for h in range(2):
                nc.scalar.activation(mvt[:, h * 384:(h + 1) * 384],
                                     psm[h][0:1, 0:384], G.Copy,
                                     scale=1.0 / D_MODEL)
            topk_scatter(mvt, LD, TK_D, s2rbuf.ap()[s:s + 1], ioL)
            Ms = []
            for tb in range(nvb):
                pw = min(128, S - tb * 128)
                Mt = tp.tile([128, LD], f32, tag=f"sl768_{tb}",
                             name=f"dMs_{tb}")
                nc.sync.dma_start(
                    Mt[0:pw, :],
                    dap(s2rbuf, s * 2 * LD + tb * 128 + 1,
                        [[1, pw], [1, LD]]))
                Ms.append(Mt)
            aggT = [tp.tile([128, LD], f32, tag=f"aagT_{j}",
                            name=f"daggT_{j}") for j in range(2)]
            for j in range(2):
                for h in range(2):
                    ps = pbig.tile([128, 512], f32, tag="ps", name="psda")
                    for tb in range(nvb):
                        pw = min(128, S - tb * 128)
                        nc.tensor.matmul(ps[:, 0:384],
                                         vn[tb][0:pw, j * 128:(j + 1) * 128],
                                         Ms[tb][0:pw, h * 384:(h + 1) * 384],
                                         start=(tb == 0),
                                         stop=(tb == nvb - 1))
                    nc.vector.tensor_copy(
                        aggT[j][:, (1 - h) * 384:(2 - h) * 384],
                        rev_free(ps[:, 0:384], 384))
            xo_tag = "xa" if kv_enc else "xb"
            xo = [sp.tile([128, LD + 2], f32, tag=f"{xo_tag}{j}",
                          name=f"dxo{j}") for j in range(2)]
            for j in range(2):
                for h in range(2):
                    ps = pbig.tile([128, 512], f32, tag="ps", name="psdo")
                    for cb in range(2):
                        nc.tensor.matmul(ps[:, 0:384],
                                         wo[cb][:, j * 128:(j + 1) * 128],
                                         aggT[cb][:, h * 384:(h + 1) * 384],
                                         start=(cb == 0), stop=False)
                    nc.tensor.matmul(ps[:, 0:384],
                                     brow[3][:, j * 128:(j + 1) * 128],
                                     onesr[:, h * 384:(h + 1) * 384],
                                     start=False, stop=True)
                    nc.vector.tensor_add(xo[j][:, h * 384:(h + 1) * 384],
                                         ps[:, 0:384],
                                         xq[j][:, h * 384:(h + 1) * 384])
            return xo

        dtr = [wload(f"dec_tr{k}", wp, f"wtr{k}") for k in range(3)]
        pjt = wload("proj", wp, "wpj")
        lnwD = wload("dec_lnw", tp, "lnwd")[0]
        lnbD = wload("dec_lnb", tp, "lnbd")[0]

        for s in range(BPC):
            # embedding
            dtok = [wload(f"dec_tok{k}", tp, f"tok{k}") for k in range(3)]
            dtim = wload("dec_time", tp, "tim")
            x0T = [sp.tile([128, LD + 2], f32, tag=f"xa{j}", name=f"x0T{j}")
                   for j in range(2)]
            spad = [tp.tile([pw, 51], f32, tag=f"spad_{p0}",
                            name=f"spad_{p0}") for p0, pw in CB321]
            for bi, (p0, pw) in enumerate(CB321):
                nc.vector.memset(spad[bi][:], 0.0)
                nc.vector.tensor_copy(spad[bi][:, 1:49],
                                      seasT[s][bi][:, 48:96])
            for j in range(2):
                for h in range(2):
                    ps = pbig.tile([128, 512], f32, tag="ps", name="psdt")
                    nc.tensor.matmul(
                        ps[:, 0:384], dtim[0][:, j * 128:(j + 1) * 128],
                        marks[s][:, SEQ_LEN + h * 384:SEQ_LEN + (h + 1) * 384],
                        start=True, stop=True)
                    nc.vector.tensor_copy(x0T[j][:, h * 384:(h + 1) * 384],
                                          ps[:, 0:384])
                ps2 = ptr.tile([128, 321], f32, tag="pst", name="psc49")
                first = True
                for k in range(3):
                    for bi in range(3):
                        nc.tensor.matmul(ps2[:, 0:49],
                                         dtok[k][bi][:, j * 128:(j + 1) * 128],
                                         spad[bi][:, k:k + 49],
                                         start=first,
                                         stop=(k == 2 and bi == 2))
                        first = False
                nc.vector.tensor_add(x0T[j][:, 0:49], ps2[:, 0:49],
                                     x0T[j][:, 0:49])
                ps3 = ptr.tile([128, 321], f32, tag="pst", name="psc1")
                for bi in range(3):
                    nc.tensor.matmul(ps3[:, 0:1],
                                     dtok[2][bi][:, j * 128:(j + 1) * 128],
                                     seasT[s][bi][:, 48:49],
                                     start=(bi == 0), stop=(bi == 2))
                nc.vector.tensor_add(x0T[j][:, 767:768], ps3[:, 0:1],
                                     x0T[j][:, 767:768])

            x1T = acl(x0T, False, s, "dec_s", "dec_sb", gd_d, sd_d)
            x1s = [sp.tile([128, LD + 2], f32, tag=f"xc{j}", name=f"x1s{j}")
                   for j in range(2)]
            t12 = [sp.tile([128, LD + 2], f32, tag=f"xt{j}", name=f"t12{j}")
                   for j in range(2)]
            dec_decomp(x1T, x1s, t12, "copy")
            x2T = acl(x1s, True, s, "dec_c", "dec_cb", gc_d, sc_d)
            x2s = [sp.tile([128, LD + 2], f32, tag=f"xd{j}", name=f"x2s{j}")
                   for j in range(2)]
            dec_decomp(x2T, x2s, t12, "add")
            # FFN
            w1 = wload("dec_f1", tp, "w1")
            w2 = wload("dec_f2", tp, "w2")
            y1 = [tp.tile([128, LD], f32, tag=f"sl768_{m}", name=f"y1D_{m}")
                  for m in range(8)]
            for m in range(8):
                for h in range(2):
                    ps = pbig.tile([128, 512], f32, tag="ps", name="psdf1")
                    for cb in range(2):
                        nc.tensor.matmul(ps[:, 0:384],
                                         w1[cb][:, m * 128:(m + 1) * 128],
                                         x2s[cb][:, h * 384:(h + 1) * 384],
                                         start=(cb == 0), stop=(cb == 1))
                    nc.scalar.activation(y1[m][:, h * 384:(h + 1) * 384],
                                         ps[:, 0:384], G.Gelu)
            x3T = [sp.tile([128, LD + 2], f32, tag=f"xb{j}", name=f"x3T{j}")
                   for j in range(2)]
            for j in range(2):
                for h in range(2):
                    ps = pbig.tile([128, 512], f32, tag="ps", name="psdf2")
                    for m in range(8):
                        nc.tensor.matmul(ps[:, 0:384],
                                         w2[m][:, j * 128:(j + 1) * 128],
                                         y1[m][:, h * 384:(h + 1) * 384],
                                         start=(m == 0), stop=(m == 7))
                    nc.vector.tensor_add(x3T[j][:, h * 384:(h + 1) * 384],
                                         ps[:, 0:384],
                                         x2s[j][:, h * 384:(h + 1) * 384])
            x3s = [sp.tile([128, LD + 2], f32, tag=f"xc{j}", name=f"x3s{j}")
                   for j in range(2)]
            dec_decomp(x3T, x3s, t12, "add")
            mylayernorm(x3s, LD, lnwD, lnbD, [(0, LD)])
            # t123 circular pad (in-place shift into cols 0..769)
            t123p = [sp.tile([128, LD + 2], f32, tag=f"xp{j}",
                             name=f"t123p{j}") for j in range(2)]
            for j in range(2):
                nc.vector.tensor_copy(t123p[j][:, 1:LD + 1], t12[j][:, 0:LD])
                nc.vector.tensor_copy(t123p[j][:, 0:1],
                                      t12[j][:, LD - 1:LD])
                nc.vector.tensor_copy(t123p[j][:, LD + 1:LD + 2],
                                      t12[j][:, 0:1])
            psa = ptr.tile([128, 321], f32, tag="pst", name="psarow")
            nc.tensor.matmul(psa[:, 0:N_SERIES], ones1[:], arow[s][:],
                             start=True, stop=True)
            abc = tp.tile([128, N_SERIES], f32, tag="abc", name="abc")
            nc.vector.tensor_copy(abc[:], psa[:, 0:N_SERIES])
            for q in range(6):
                base = LABEL_LEN + q * 120
                pso = pout.tile([120, N_SERIES], f32, tag="pso", name="pso")
                nc.tensor.matmul(pso[:], x3s[0][:, base:base + 120],
                                 pjt[0][:], start=True, stop=False)
                nc.tensor.matmul(pso[:], x3s[1][:, base:base + 120],
                                 pjt[1][:], start=False, stop=False)
                for k in range(3):
                    for j in range(2):
                        nc.tensor.matmul(pso[:],
                                         t123p[j][:, base + k:base + k + 120],
                                         dtr[k][j][:], start=False,
                                         stop=(k == 2 and j == 1))
                ot = tp.tile([120, N_SERIES], f32, tag="otq", name="otq")
                nc.vector.tensor_add(ot[:], pso[:], abc[0:120, :])
                oab = tp.tile([120, N_SERIES], f32, tag="oab", name="oab")
                nc.scalar.activation(oab[:], ot[:], G.Abs)
                rmx = tp.tile([120, 1], f32, tag="rmx", name="rmx")
                nc.vector.tensor_reduce(rmx[:], oab[:], X, A.max)
                sc = tp.tile([120, 1], f32, tag="scq", name="scq")
                nc.vector.tensor_scalar(sc[:], rmx[:], 1.0 / 127.0, 1e-30,
                                        A.mult, A.add)
                rs = tp.tile([120, 1], f32, tag="rsq", name="rsq")
                nc.vector.reciprocal(rs[:], sc[:])
                oq = tp.tile([120, N_SERIES], i8, tag="oqq", name="oqq")
                nc.vector.tensor_scalar_mul(oq[:], ot[:], rs[:])
                nc.sync.dma_start(out_d.ap()[s, q * 120:(q + 1) * 120],
                                  oq[:])
                nc.sync.dma_start(osc_d.ap()[s, q * 120:(q + 1) * 120],
                                  sc[:])
    nc.compile()
    return nc
